# revision 1
# baseline (speedup 1.0000x reference)
from collections import deque

import numpy as np

import concourse.mybir as mybir
from concourse import bacc
from concourse.tile import TileContext
from concourse.bass_utils import run_bass_kernel_spmd

H, D, DH = 12, 768, 64
B, S = 2, 2048
NCORES = 8
CORES_PER_BATCH = 4
HPC = 3
SQ = 512
NSQ = S // SQ
NSK = S // 128
NDC = D // 128

F32 = mybir.dt.float32
F32R = mybir.dt.float32r
F16 = mybir.dt.float16
ADD = mybir.AluOpType.add
MULT = mybir.AluOpType.mult
EXP = mybir.ActivationFunctionType.Exp


def _build_module():
    nc = bacc.Bacc("TRN2", target_bir_lowering=False, debug=False, num_devices=NCORES)
    xT = nc.declare_dram_parameter("xT", [128, NDC, S], F32R, isOutput=False)
    wqk = nc.declare_dram_parameter("wqk", [128, HPC, NDC, 128], F32R, isOutput=False)
    wv = nc.declare_dram_parameter("wv", [128, NDC, 256], F32R, isOutput=False)
    wo01 = nc.declare_dram_parameter("wo01", [128, D], F32R, isOutput=False)
    wo2 = nc.declare_dram_parameter("wo2", [64, D], F32R, isOutput=False)
    bqk = nc.declare_dram_parameter("bqk", [128, HPC], F32, isOutput=False)
    bv = nc.declare_dram_parameter("bv", [128, HPC * DH], F32, isOutput=False)
    out = nc.declare_dram_parameter("out", [S, D], F32, isOutput=True)

    with TileContext(nc) as tc:
        _body(nc, tc, xT, wqk, wv, wo01, wo2, bqk, bv, out)
    nc.compile()
    return nc


def _body(nc, tc, xT, wqk, wv, wo01, wo2, bqk, bv, out):
    with (
        tc.tile_pool(name="persist", bufs=1) as P1,
        tc.tile_pool(name="work", bufs=4) as W2,
        tc.tile_pool(name="probs", bufs=2) as PR,
        tc.tile_pool(name="acc", bufs=4, space="PSUM") as ACC,
        tc.tile_pool(name="sps", bufs=2, space="PSUM") as SPS,
    ):
        xT_sb = P1.tile([128, NDC, S], F32R, tag="xT")
        wqk_sb = P1.tile([128, HPC, NDC, 128], F32R, tag="wqk")
        wv_sb = P1.tile([128, NDC, 256], F32R, tag="wv")
        wo01_sb = P1.tile([128, D], F32R, tag="wo01")
        wo2_sb = P1.tile([64, D], F32R, tag="wo2")
        bqk_sb = P1.tile([128, HPC], F32, tag="bqk")
        bv_sb = P1.tile([128, HPC * DH], F32, tag="bv")
        qT = [
            P1.tile([64, S], F32R, tag=f"qT{h}", name=f"qT{h}")
            for h in range(HPC)
        ]
        kT = [
            P1.tile([64, S], F32R, tag=f"kT{h}", name=f"kT{h}")
            for h in range(HPC)
        ]
        vp = P1.tile([128, NSK, HPC * 128], F16, tag="vp")

        nc.sync.dma_start(xT_sb[:, 0, 0:SQ], xT[:, 0, 0:SQ])
        nc.sync.dma_start(wqk_sb[:, 0, :, :], wqk[:, 0, :, :])
        for o in range(1, NDC):
            nc.sync.dma_start(xT_sb[:, o, 0:SQ], xT[:, o, 0:SQ])
        nc.sync.dma_start(bqk_sb[:], bqk[:])
        for sc in range(1, NSQ):
            nc.sync.dma_start(
                xT_sb[:, :, sc * SQ:(sc + 1) * SQ], xT[:, :, sc * SQ:(sc + 1) * SQ]
            )
        nc.sync.dma_start(wv_sb[:], wv[:])
        nc.sync.dma_start(wqk_sb[:, 1:3, :, :], wqk[:, 1:3, :, :])
        nc.sync.dma_start(bv_sb[:], bv[:])
        nc.sync.dma_start(wo01_sb[:], wo01[:])
        nc.sync.dma_start(wo2_sb[:], wo2[:])
        nc.gpsimd.memset(
            vp[:].rearrange("p s (h m) -> p s h m", m=128)[:, :, :, 64:128], 1.0
        )

        def qk_unit(h, sc):
            ps = ACC.tile([128, SQ], F32, tag="acc", name=f"qkps{h}_{sc}")
            for o in range(NDC):
                nc.tensor.matmul(
                    ps[:],
                    wqk_sb[:, h, o, :],
                    xT_sb[:, o, sc * SQ:(sc + 1) * SQ],
                    start=(o == 0),
                    stop=(o == NDC - 1),
                )
            nc.vector.tensor_tensor(
                qT[h][:, sc * SQ:(sc + 1) * SQ],
                ps[0:64, :],
                bqk_sb[0:64, h:h + 1].to_broadcast([64, SQ]),
                ADD,
            )
            nc.vector.tensor_tensor(
                kT[h][:, sc * SQ:(sc + 1) * SQ],
                ps[64:128, :],
                bqk_sb[64:128, h:h + 1].to_broadcast([64, SQ]),
                ADD,
            )

        def v_unit(sc):
            ps = ACC.tile([128, 256], F32, tag="acc", name=f"vps{sc}")
            for o in range(NDC):
                nc.tensor.matmul(
                    ps[:],
                    xT_sb[:, o, sc * 128:(sc + 1) * 128],
                    wv_sb[:, o, :],
                    start=(o == 0),
                    stop=(o == NDC - 1),
                )
            nc.vector.tensor_tensor(
                vp[:, sc, :].rearrange("p (h m) -> p h m", m=128)[:, :, 0:64],
                ps[:, 0:HPC * 64].rearrange("p (h m) -> p h m", m=64),
                bv_sb[:].rearrange("p (h m) -> p h m", m=64),
                ADD,
            )

        def proj_stage1(sc, ms, ctx01, store):
            tiles = []
            for n0, nw in ((0, 512), (512, 256)):
                ops_t = ACC.tile([128, nw], F32, tag="acc", name=f"ops{sc}_{ms}_{n0}")
                nc.tensor.matmul(
                    ops_t[:],
                    ctx01[:, ms * 128:(ms + 1) * 128],
                    wo01_sb[:, n0:n0 + nw],
                    start=True,
                    stop=False,
                )
                tiles.append((n0, nw, ops_t))
            store[ms] = tiles

        def proj_stage2(sc, ms, ctx2, store):
            ot = W2.tile([128, D], F32, tag="out", name=f"ot{sc}_{ms}")
            for n0, nw, ops_t in store.pop(ms):
                nc.tensor.matmul(
                    ops_t[:],
                    ctx2[:, ms * 128:(ms + 1) * 128],
                    wo2_sb[:, n0:n0 + nw],
                    start=False,
                    stop=True,
                )
                nc.vector.tensor_copy(ot[:, n0:n0 + nw], ops_t[:])
            nc.sync.dma_start(
                out[(sc * 4 + ms) * 128:(sc * 4 + ms + 1) * 128, :], ot[:]
            )

        def proj_unit(sc, ms, ctx01, ctx2):
            store = {}
            proj_stage1(sc, ms, ctx01, store)
            proj_stage2(sc, ms, ctx2, store)

        filler = deque()
        stores = {}

        def attention_block(sc, h, ctx01, ctx2, pops_per_j=1, pop_stride=1):
            probs = PR.tile([128, NSK * SQ], F16, tag="probs", name=f"pr{sc}_{h}")
            cps = ACC.tile([128, SQ], F32, tag="acc", name=f"cps{sc}_{h}")

            def probsv(mk):
                nc.tensor.matmul(
                    cps[:],
                    vp[:, mk, h * 128:(h + 1) * 128],
                    probs[:, mk * SQ:(mk + 1) * SQ],
                    start=(mk == 0),
                    stop=(mk == NSK - 1),
                )

            for j in range(NSK // 2):
                sps = SPS.tile([128, 2 * SQ], F32, tag="sps", name=f"sps{sc}_{h}_{j}")
                for half in range(2):
                    mk = 2 * j + half
                    nc.tensor.matmul(
                        sps[:, half * SQ:(half + 1) * SQ],
                        kT[h][:, mk * 128:(mk + 1) * 128],
                        qT[h][:, sc * SQ:(sc + 1) * SQ],
                        start=True,
                        stop=True,
                    )
                nc.scalar.activation(
                    probs[:, j * 2 * SQ:(j + 1) * 2 * SQ], sps[:], EXP,
                    scale=0.125,
                )
                if j % pop_stride == 0:
                    for _ in range(pops_per_j):
                        if filler:
                            filler.popleft()()
                if j > 0:
                    probsv(2 * (j - 1))
                    probsv(2 * j - 1)
            probsv(NSK - 2)
            probsv(NSK - 1)
            r = W2.tile([64, SQ], F32, tag="recip", name=f"r{sc}_{h}")
            nc.vector.reciprocal(r[:], cps[64:128, :])
            dst = ctx01[h * 64:(h + 1) * 64, :] if h < 2 else ctx2[:]
            nc.vector.tensor_tensor(dst, cps[0:64, :], r[:], MULT)

        warm = P1.tile([64, 512], F32R, tag="warm")
        nc.vector.memset(warm[:].bitcast(F32), 0.0)
        wps = ACC.tile([128, 512], F32, tag="acc", name="warmps")
        for _ in range(10):
            nc.tensor.matmul(wps[:], warm[:, 0:128], warm[:], start=True, stop=True)
        wact = P1.tile([64, 1], F16, tag="wact")
        nc.scalar.activation(wact[:], warm[:, 0:1].bitcast(F32), EXP, scale=0.125)

        qk_unit(0, 0)

        ctxs = {}
        for sc in range(NSQ):
            ctxs[sc] = (
                W2.tile([128, SQ], F32R, tag="ctx01", name=f"c01_{sc}"),
                W2.tile([64, SQ], F32R, tag="ctx2", name=f"c2_{sc}"),
            )
            stores.setdefault(sc, {})
            for h in range(HPC):
                pops = 1
                if sc == 0 and h == 0:
                    filler.append(lambda: qk_unit(0, 1))
                    filler.append(lambda: qk_unit(0, 2))
                    filler.append(lambda: qk_unit(0, 3))
                    for i in range(NSK // 2):
                        filler.append(lambda i=i: v_unit(2 * i))
                        filler.append(lambda i=i: v_unit(2 * i + 1))
                        if i % 2 == 0:
                            filler.append(lambda i=i: qk_unit(1, i // 2))
                    pops = 3
                elif sc == 0 and h == 1:
                    for i in range(NSQ):
                        filler.append(lambda i=i: qk_unit(2, i))
                elif sc == NSQ - 1 and h == HPC - 1:
                    filler.append(
                        lambda: proj_stage1(sc, 0, ctxs[sc][0], stores[sc])
                    )
                stride = 3 if sc > 0 else (3 if h == 1 else 1)
                attention_block(sc, h, *ctxs[sc], pops_per_j=pops,
                                pop_stride=stride)
            for ms in range(SQ // 128):
                if sc == NSQ - 1 and ms == 0:
                    filler.append(
                        lambda sc=sc: proj_stage2(sc, 0, ctxs[sc][1], stores[sc])
                    )
                    continue
                filler.append(
                    lambda sc=sc, ms=ms: proj_stage1(sc, ms, ctxs[sc][0], stores[sc])
                )
                filler.append(
                    lambda sc=sc, ms=ms: proj_stage2(sc, ms, ctxs[sc][1], stores[sc])
                )
        while filler:
            filler.popleft()()


_CACHE = {}


def _get_module():
    if "nc" not in _CACHE:
        _CACHE["nc"] = _build_module()
    return _CACHE["nc"]


def make_in_maps(x, Wq, Wk, Wv, bq, bk, bv, Wo):
    f = np.float32
    in_maps = []
    for c in range(NCORES):
        b = c // CORES_PER_BATCH
        hh = [HPC * (c % CORES_PER_BATCH) + i for i in range(HPC)]
        xt = x[b].T.reshape(NDC, 128, S).transpose(1, 0, 2)
        wqk = np.stack(
            [np.concatenate([Wq[h], Wk[h]], axis=1) for h in hh]
        )
        wqk = wqk.reshape(HPC, NDC, 128, 128).transpose(2, 0, 1, 3)
        wv_stack = np.concatenate(
            [Wv[h] for h in hh] + [np.zeros((D, 64), f)], axis=1
        )
        wv_stack = wv_stack.reshape(NDC, 128, 256).transpose(1, 0, 2)
        in_maps.append({
            "xT": np.ascontiguousarray(xt).astype(f, copy=False),
            "wqk": np.ascontiguousarray(wqk).astype(f, copy=False),
            "wv": np.ascontiguousarray(wv_stack).astype(f, copy=False),
            "wo01": np.ascontiguousarray(Wo[hh[0] * DH:(hh[0] + 2) * DH, :]).astype(f, copy=False),
            "wo2": np.ascontiguousarray(Wo[hh[2] * DH:(hh[2] + 1) * DH, :]).astype(f, copy=False),
            "bqk": np.ascontiguousarray(
                np.stack([np.concatenate([bq[h], bk[h]]) for h in hh], axis=1)
            ).astype(f, copy=False),
            "bv": np.ascontiguousarray(
                np.tile(np.concatenate([bv[h] for h in hh]), (128, 1))
            ).astype(f, copy=False),
        })
    return in_maps


def gather(results, bo):
    out = np.empty((B, S, D), np.float32)
    for b in range(B):
        acc = results[b * CORES_PER_BATCH]["out"].astype(np.float32, copy=True)
        for c in range(b * CORES_PER_BATCH + 1, (b + 1) * CORES_PER_BATCH):
            acc += results[c]["out"]
        out[b] = acc + bo[None, :].astype(np.float32)
    return out


def kernel(x, Wq, Wk, Wv, bq, bk, bv, Wo, bo, c=0, **_unused):
    x, Wq, Wk, Wv, bq, bk, bv, Wo, bo = (
        np.asarray(a, np.float32) for a in (x, Wq, Wk, Wv, bq, bk, bv, Wo, bo)
    )
    nc = _get_module()
    in_maps = make_in_maps(x, Wq, Wk, Wv, bq, bk, bv, Wo)
    res = run_bass_kernel_spmd(nc, in_maps, list(range(NCORES)))
    return gather(res.results, bo)

